# revision 1
# baseline (speedup 1.0000x reference)
"""Trainium2 Bass kernel for nn_AutoregressiveBisectionInverter (v10).

Closed-form cubic root per autoregressive step: solve v^3+v = nd via
v = (t^(1/3) - t^(-1/3))/sqrt(3), t = z + sqrt(z^2+1), z = (3sqrt3/2)nd;
x_k = sat_k * v. With E' = sat*e^(ln(t)/3) (sat folded into Exp's bias),
x_k = E' - sat^2/E', so the second Exp of earlier versions becomes a DVE
reciprocal + one fused tensor_scalar.

Drain-free semaphore-edge schedule, one wait per instruction, transitive
coverage through the in-order engines:

ACT (6 ops/step, 342ns issue):
  tanh[w sV=cbfix_k]
  -> z=Identity[w sA=tanh_k] -> Square(no wait; z's gate covers tanh)
  -> Sqrt[w sA=sq] -> Ln(bias=z)[w sA=sqrt] -> E'=Exp(L/3+lnsat)[w sA=ln]

DVE per iteration k: rcp=1/E' [w sA=E_k, transitively covers the whole
step] -> x_k = -sat^2*rcp + E' [w sV=rcp] -> cbfix_{k+1} =
W[k+1,k-1]x_{k-1}+cba [w sV=its cba producer, a formal RAW edge past the
wide-op ack window] -> paired prefix dots (rows kk,kk+1 in one [128,2,m]
mult+reduce) -> special tensor_scalar chains for the first rows.
tanh_k waits d_cbfix[k], which transitively covers x_{k-1}'s retirement.
"""

import numpy as np

B, D = 1024, 32
NCORES = 8
ROWS = B // NCORES  # 128 rows per core == SBUF partitions
NSPEC = 5           # prefix dots for kk=2..NSPEC via tensor_scalar chains
ZSET = {2, 6, 10, 14, 18, 22}  # steps whose z runs on DVE


def _softplus64(x):
    x = x.astype(np.float64)
    return np.log1p(np.exp(-np.abs(x))) + np.maximum(x, 0)


def build(y, W, s, b):
    """Build the SPMD Bass program; returns (nc, in_maps)."""
    from contextlib import ExitStack
    import concourse.bass as bass
    from concourse import mybir

    f32 = mybir.dt.float32
    Alu = mybir.AluOpType
    Act = mybir.ActivationFunctionType

    y = np.ascontiguousarray(np.asarray(y), dtype=np.float32)
    W64 = np.asarray(W, dtype=np.float64)
    s64 = np.asarray(s, dtype=np.float64)
    b64 = np.asarray(b, dtype=np.float64)

    # ---- host precompute (elementwise input normalization only) ----
    abar = 10.0 * _softplus64(s64)
    sqrt_abar = np.sqrt(abar)
    kappa = 10.0 * abar ** -1.5
    CC = 3.0 * np.sqrt(3.0) / 2.0
    kz = (CC * kappa).astype(np.float32)
    Yz = (CC * 10.0 * y.astype(np.float64) * abar[None, :] ** -1.5).astype(np.float32)
    sat64 = sqrt_abar / np.sqrt(3.0)
    lnsat = np.log(sat64).astype(np.float32)
    sat2 = (sat64 * sat64).astype(np.float32)
    Wq = W64.astype(np.float32)          # weights on x are original W
    c0 = float(-kz[0] * np.tanh(b64[0]))
    bt1 = float(b64[1])

    # wpy row for kk holds [b_kk, W[kk,0..kk-3], 0...]; rows kk-2 and kk-1
    # are sliced together for the paired dots. Only rows kk=6..D-1 are read
    # (the specials cover kk<=NSPEC), so the table starts at row 4.
    NWPY = D - 2 - 4
    WPY = np.zeros((NWPY, D), np.float32)
    for k in range(6, D):
        WPY[k - 6, 0] = b64[k]
        WPY[k - 6, 1:k - 1] = Wq[k, 0:k - 2]
    WPYB = np.ascontiguousarray(np.broadcast_to(WPY[None], (ROWS, NWPY, D)))

    # hdr columns: [ one | c0 | bt1 | b_2..b_NSPEC | yzA(NA) | lnsA(NA) |
    #                yzB | lnsB ]  (A-part lands in the first small DMA)
    NA = 10
    NCST = 3 + (NSPEC - 1)
    HW = NCST + 2 * D
    LNSB = np.broadcast_to(lnsat[None, :], (ROWS, D))

    nc = bass.Bass()
    hd_d = nc.dram_tensor("hdr", [ROWS, HW], f32, kind="ExternalInput")
    wp_d = nc.dram_tensor("wpy", [ROWS, NWPY, D], f32, kind="ExternalInput")
    xo_d = nc.dram_tensor("xout", [ROWS, D], f32, kind="ExternalOutput")

    with ExitStack() as ctx:
        vx = ctx.enter_context(nc.sbuf_tensor([ROWS, D + 1], f32))  # [1, x_0..]
        hdr = ctx.enter_context(nc.sbuf_tensor([ROWS, HW], f32))
        onec = hdr[:, 0:1]
        c0c = hdr[:, 1:2]
        bt1c = hdr[:, 2:3]
        bsc = hdr[:, 3:3 + (NSPEC - 1)]

        def yzc(k):
            c = NCST + k if k < NA else NCST + 2 * NA + (k - NA)
            return hdr[:, c:c + 1]

        def lnsc(k):
            c = NCST + NA + k if k < NA else NCST + NA + D + (k - NA)
            return hdr[:, c:c + 1]

        wpy = ctx.enter_context(nc.sbuf_tensor([ROWS, NWPY, D], f32))
        prod = ctx.enter_context(nc.sbuf_tensor([ROWS, 2, D], f32))
        tt = ctx.enter_context(nc.sbuf_tensor([ROWS, 1], f32))
        z2 = ctx.enter_context(nc.sbuf_tensor([ROWS, 1], f32))
        rr = ctx.enter_context(nc.sbuf_tensor([ROWS, 1], f32))
        ll = ctx.enter_context(nc.sbuf_tensor([ROWS, 1], f32))
        ee = ctx.enter_context(nc.sbuf_tensor([ROWS, 1], f32))
        rcp = ctx.enter_context(nc.sbuf_tensor([ROWS, 1], f32))
        zz = ctx.enter_context(nc.sbuf_tensor([ROWS, 1], f32))
        cb = ctx.enter_context(nc.sbuf_tensor([ROWS, 2], f32))
        # cba: [0:4] two double-col buffers for paired dots (by pair parity);
        # [4:4+NSPEC-2] dedicated cols for the special chains kk=3..NSPEC
        cba = ctx.enter_context(nc.sbuf_tensor([ROWS, 4 + NSPEC - 2], f32))
        s_dma = ctx.enter_context(nc.semaphore("s_dma"))
        sA = ctx.enter_context(nc.semaphore("sA"))    # ACT chain counter
        sV = ctx.enter_context(nc.semaphore("sV"))    # DVE chain counter
        block = ctx.enter_context(nc.Block())

        # ---- pre-pass: compute every sem-count landmark ----
        a_tanh, a_sq, a_sqrt, a_ln, a_e = {}, {}, {}, {}, {}
        pa = 0
        for k in range(D):
            if k >= 1:
                pa += 1
                a_tanh[k] = pa
            if not (k in ZSET and k != NA - 1 and k >= 1):
                pa += 1  # z on ACT (1-in-3 steps compute z on DVE)
            pa += 1
            a_sq[k] = pa
            pa += 1
            a_sqrt[k] = pa
            pa += 1
            a_ln[k] = pa
            pa += 1
            a_e[k] = pa

        # DVE emission order per iteration k: [wpy-gate nop at k=4]
        #   rcp_k | subx_k | cbfix_{k+1} | pair(kk=k+2) | specials
        d_sub, d_rcp, d_red, d_cbfix, d_spec, dz = {}, {}, {}, {}, {}, {}
        pd = 1  # memset
        for k in range(D):
            if k == 4:
                pd += 1  # wpyA-gate nop
            if k == 13:
                pd += 1  # wpyB-gate nop
            pd += 1
            d_rcp[k] = pd
            pd += 1
            d_sub[k] = pd
            if 2 <= k + 1 <= D - 1:
                pd += 1  # cbfix_{k+1}
                d_cbfix[k + 1] = pd
            if k + 1 <= D - 1 and (k + 1) in ZSET and k + 1 != NA - 1 and k + 1 >= 1:
                pd += 1  # z_{k+1} on DVE (1-in-3 steps)
                dz[k + 1] = pd
            kk = k + 2
            if kk >= NSPEC + 1 and kk % 2 == 0 and kk <= D - 2:
                pd += 1  # mult2
                pd += 1
                d_red[kk] = pd
            for kk2 in range(k + 3, NSPEC + 1):
                pd += 1
                d_spec[kk2] = pd

        @block.scalar
        def _(scalar):
            for k in range(D):
                # tanh_k (k=0: T_0=tanh(b_0) folded into c0)
                if k == 1:
                    nc.scalar.activation(
                        out=tt[:, :], in_=vx[:, 1:2], func=Act.Tanh,
                        bias=bt1c[:, :], scale=float(Wq[1, 0]))._wait_ge(
                            sV, d_sub[0]).then_inc(sA, 1)
                elif k >= 2:
                    nc.scalar.activation(
                        out=tt[:, :], in_=vx[:, k:k + 1], func=Act.Tanh,
                        bias=cb[:, k % 2:k % 2 + 1],
                        scale=float(Wq[k, k - 1]))._wait_ge(
                            sV, d_cbfix[k]).then_inc(sA, 1)
                # z_k = -kz*T + yz (Ln's bias operand); odd steps compute
                # it on DVE instead to balance the two issue queues
                if k == 0:
                    nc.scalar.activation(
                        out=zz[:, :], in_=yzc(0), func=Act.Identity,
                        bias=c0c[:, :], scale=1.0)._wait_ge(
                            s_dma, 16).then_inc(sA, 1)
                elif not (k in ZSET and k != NA - 1):
                    nc.scalar.activation(
                        out=zz[:, :], in_=tt[:, :], func=Act.Identity,
                        bias=yzc(k), scale=float(-kz[k]))._wait_ge(
                            sA, a_tanh[k]).then_inc(sA, 1)
                # Square: even k rides z's gate; odd k waits DVE's z_k
                if k == 0:
                    nc.scalar.activation(
                        out=z2[:, :], in_=yzc(0), func=Act.Square,
                        bias=c0c[:, :], scale=1.0).then_inc(sA, 1)
                else:
                    inst = nc.scalar.activation(
                        out=z2[:, :], in_=tt[:, :], func=Act.Square,
                        bias=yzc(k), scale=float(-kz[k]))
                    if k == NA - 1:
                        inst._wait_ge(s_dma, 48)  # hdrB gate (z_9 on ACT)
                    elif k in ZSET:
                        inst._wait_ge(sV, dz[k])
                    inst.then_inc(sA, 1)
                nc.scalar.activation(
                    out=rr[:, :], in_=z2[:, :], func=Act.Sqrt,
                    bias=onec[:, :], scale=1.0)._wait_ge(
                        sA, a_sq[k]).then_inc(sA, 1)
                nc.scalar.activation(
                    out=ll[:, :], in_=rr[:, :], func=Act.Ln, bias=zz[:, :],
                    scale=1.0)._wait_ge(sA, a_sqrt[k]).then_inc(sA, 1)
                nc.scalar.activation(
                    out=ee[:, :], in_=ll[:, :], func=Act.Exp,
                    bias=lnsc(k), scale=float(1.0 / 3.0))._wait_ge(
                        sA, a_ln[k]).then_inc(sA, 1)

        @block.vector
        def _(vector):
            nc.vector.memset(vx[:, 0:1], 1.0).then_inc(sV, 1)
            for k in range(D):
                if k == 4:
                    # wpyA-gate: pairs kk=6..14 see the wpyA DMA done
                    nc.vector.memset(cba[:, 0:1], 0.0)._wait_ge(
                        s_dma, 32).then_inc(sV, 1)
                if k == 13:
                    # wpyB-gate: pairs kk>=16 see the wpyB DMA done
                    nc.vector.memset(cba[:, 0:1], 0.0)._wait_ge(
                        s_dma, 64).then_inc(sV, 1)
                # rcp_k = 1/E'  (its wait transitively implies everything
                # through tanh_k, incl. subx_{k-1} and cbfix_k)
                nc.vector.reciprocal(out=rcp[:, :], in_=ee[:, :])._wait_ge(
                    sA, a_e[k]).then_inc(sV, 1)
                # x_k = -sat^2*rcp + E'
                nc.vector.tensor_scalar(
                    out=vx[:, k + 1:k + 2], in0=rcp[:, :],
                    scalar1=float(-sat2[k]), scalar2=ee[:, 0:1],
                    op0=Alu.mult, op1=Alu.add)._wait_ge(
                        sV, d_rcp[k]).then_inc(sV, 1)
                # cbfix_{k+1}: cb = W[k+1,k-1]*x_{k-1} + cba  (k+1 in 2..D-1)
                # single wait = its cba producer (RAW edge past the wide-op
                # ack window); x_{k-1} covered transitively via rcp's wait.
                kk1 = k + 1
                if 2 <= kk1 <= D - 1:
                    if kk1 == 2:
                        cba_src = bsc[:, 0:1]
                        w_cba = None
                    elif kk1 <= NSPEC:
                        cba_src = cba[:, kk1 + 1:kk1 + 2]  # special col
                        w_cba = d_spec[kk1]
                    else:
                        cp = 2 * ((kk1 // 2) % 2)
                        cba_src = cba[:, cp + (kk1 % 2):cp + (kk1 % 2) + 1]
                        w_cba = d_red[kk1] if kk1 % 2 == 0 else d_red[kk1 - 1]
                    inst = nc.vector.tensor_scalar(
                        out=cb[:, kk1 % 2:kk1 % 2 + 1], in0=vx[:, k:k + 1],
                        scalar1=float(Wq[kk1, kk1 - 2]), scalar2=cba_src,
                        op0=Alu.mult, op1=Alu.add)
                    if w_cba is not None:
                        inst._wait_ge(sV, w_cba)
                    inst.then_inc(sV, 1)
                # z_{k+1} on DVE for odd steps (queue balancing)
                if k + 1 <= D - 1 and (k + 1) in ZSET and k + 1 != NA - 1:
                    nc.vector.tensor_scalar(
                        out=zz[:, :], in0=tt[:, :],
                        scalar1=float(-kz[k + 1]), scalar2=yzc(k + 1),
                        op0=Alu.mult, op1=Alu.add)._wait_ge(
                            sA, a_tanh[k + 1]).then_inc(sV, 1)
                kk = k + 2
                if kk >= NSPEC + 1 and kk % 2 == 0 and kk <= D - 2:
                    m = kk
                    c4 = 2 * ((kk // 2) % 2)
                    a = vx[:, 0:m]
                    vxb = bass.AP(tensor=a.tensor, offset=a.offset,
                                  ap=[list(a.ap[0]), [0, 2], [1, m]])
                    nc.vector.tensor_tensor(
                        out=prod[:, 0:2, 0:m], in0=vxb,
                        in1=wpy[:, kk - 6:kk - 4, 0:m], op=Alu.mult)._wait_ge(
                            sV, d_sub[k]).then_inc(sV, 1)
                    nc.vector.tensor_reduce(
                        out=cba[:, c4:c4 + 2], in_=prod[:, 0:2, 0:m],
                        axis=mybir.AxisListType.X, op=Alu.add)._wait_ge(
                            sV, d_red[kk] - 1).then_inc(sV, 1)
                # special prefix chains (kk2=3..NSPEC): add Wq[kk2,k]*x_k
                first = True
                for kk2 in range(k + 3, NSPEC + 1):
                    src = bsc[:, kk2 - 2:kk2 - 1] if k == 0 else cba[:, kk2 + 1:kk2 + 2]
                    inst = nc.vector.tensor_scalar(
                        out=cba[:, kk2 + 1:kk2 + 2], in0=vx[:, k + 1:k + 2],
                        scalar1=float(Wq[kk2, k]), scalar2=src,
                        op0=Alu.mult, op1=Alu.add)
                    if first:
                        inst._wait_ge(sV, d_sub[k])  # x_k RAW edge
                        first = False
                    inst.then_inc(sV, 1)

        NHA = NCST + 2 * NA        # hdrA: consts + yzA + lnsA
        NWA = 10                   # wpyA rows 0..9 (pairs kk=6..14)

        @block.sync
        def _(sync):
            sync.dma_start(out=hdr[:, 0:NHA],
                           in_=hd_d[:, 0:NHA]).then_inc(s_dma, 16)
            sync.dma_start(out=wpy[:, 0:NWA, :],
                           in_=wp_d[:, 0:NWA, :]).then_inc(s_dma, 16)
            sync.dma_start(out=hdr[:, NHA:HW],
                           in_=hd_d[:, NHA:HW]).then_inc(s_dma, 16)
            sync.dma_start(out=wpy[:, NWA:NWPY, :],
                           in_=wp_d[:, NWA:NWPY, :]).then_inc(s_dma, 16)
            sync.dma_start(out=xo_d[:, :], in_=vx[:, 1:D + 1])._wait_ge(
                sV, d_sub[D - 1]).then_inc(s_dma, 16)
            sync.wait_ge(s_dma, 80)

    bs_cols = np.broadcast_to(
        b64[2:NSPEC + 1].astype(np.float32)[None, :], (ROWS, NSPEC - 1))
    in_maps = []
    for c in range(NCORES):
        yzc_ = Yz[c * ROWS:(c + 1) * ROWS]
        hdr_np = np.concatenate([
            np.full((ROWS, 1), 1.0, np.float32),
            np.full((ROWS, 1), c0, np.float32),
            np.full((ROWS, 1), bt1, np.float32),
            bs_cols,
            yzc_[:, 0:10],
            LNSB[:, 0:10],
            yzc_[:, 10:],
            LNSB[:, 10:],
        ], axis=1)
        in_maps.append({"hdr": np.ascontiguousarray(hdr_np), "wpy": WPYB})
    return nc, in_maps


def kernel(y, W, s, b):
    from concourse.bass_utils import run_bass_kernel_spmd

    nc, in_maps = build(y, W, s, b)
    res = run_bass_kernel_spmd(nc, in_maps, list(range(NCORES))).results
    X = np.concatenate([res[c]["xout"] for c in range(NCORES)], axis=0)
    return X.astype(np.float32)


if __name__ == "__main__":
    rng = np.random.default_rng(0)
    y = rng.standard_normal((B, D)).astype(np.float32)
    W = np.tril(rng.standard_normal((32, 32)), -1).astype(np.float32) * 0.5
    s = rng.standard_normal(D).astype(np.float32)
    b = rng.standard_normal(D).astype(np.float32)
    X = kernel(y=y, W=W, s=s, b=b)
    print("out", X.shape, X.dtype, X[0, :4])

